# revision 8
# baseline (speedup 1.0000x reference)
"""Trainium2 Bass kernel for the ConcreteLayer training forward pass.

Computes out = x @ softmax((weight - ln(-ln((1-tiny)*uniform + tiny))) / T, axis=1)

Strategy (8 NeuronCores, 4x2 grid):
  - 4 batch groups x 2 out-column halves; core = 2*p + q.
  - uniform is shipped as v' = (1-u) * 2^14 in fp16 (host re-encoding):
    full relative precision exactly where the gumbel is sensitive (u -> 1)
    at half the HBM bytes; the device consumes it with the same single Ln
    op via bias=1, scale=-(1-tiny)/2^14.  Measured end-to-end rel err is
    identical to the f32-u path (2.3e-3).
  - u and w live in resident SBUF tiles and their chunk DMAs are emitted
    dep-free up front on the sync queue, so no descriptor generation ever
    waits inline on a pool buffer or a collective.  Engine instruction
    streams are IN-ORDER at runtime: any DMA with an unmet semaphore
    stalls everything behind it on that engine, so CC-dependent DMAs
    (cc_in upload, cc_out readback) also ride sync, after the bulk
    descriptors, where a stall is harmless.
  - Chunk-pipelined softmax: Ln+Ln on scalar into an f32 scratch,
    (w - m) subtract on vector for the first two chunks then gpsimd,
    wide Exp on scalar, per-chunk 3D tensor_reduce row sums on vector.
  - Row sums exchanged between column-half siblings in fine-grained
    AllGather groups; finish(g) (normalize on vector + matmuls) is
    emitted one group late with an artificial dep on a later reduce so
    the vector stream never stalls on a CC round trip.
  - xt (lhsT) loads are dependency-paced behind each chunk's first Ln.
  - A dummy matmul accumulation group at t~8us spins the PE clock (HAM)
    up before the real GEMM; a dummy AllGather warms the CC stream.
  - GEMM: bf16 lhsT x normalized bf16 e, f32 PSUM, 8 banks.
  - Output stored as bf16 (upcast on host) to trim HBM store traffic.
"""

import sys

import numpy as np

for _p in ("/opt/trn_rl_repo",):
    if _p not in sys.path:
        sys.path.insert(0, _p)

B, IN, OUT = 4096, 4096, 1024
GB, GO = 4, 2  # batch groups x out-half groups
BS = B // GB  # 1024 batch rows per core
OH = OUT // GO  # 512 out cols per core
P = 128
KT = IN // P  # 32 contraction tiles
MBT = BS // P  # 8 output row tiles per core
NCORES = 8
CHUNKS = [2, 2, 4, 4, 4, 4, 4, 4, 4]  # ktiles per softmax chunk (sum 32)
VEC_SUB_CHUNKS = 2  # first N chunks do the subtract on vector (latency)
GROUPS = [2, 2, 4, 8, 8, 4, 4]  # ktiles per row-sum exchange group
XTC = 4  # ktiles per xt load chunk
N_WARM_MM = 32  # dummy matmuls to spin up the PE clock
VSCALE = 16384.0  # 2^14 host pre-scale on v' = 1-u
TINY = float(np.finfo(np.float32).tiny)

_PROGRAM = None
LAST_RESULT = None


def _pin_act_tables():
    """Steer the act-table-load pass to one set (has both Ln and Exp) so the
    compiler emits one ACT_TABLE_LOAD instead of reloading per tile."""
    import concourse.mybir as mybir
    from concourse import bacc, hw_specs

    orig = hw_specs.get_activation_tables.__wrapped__
    target = "natural_log_exp_and_others"
    strip = {
        mybir.ActivationFunctionType.Ln,
        mybir.ActivationFunctionType.Exp,
    }

    def pinned(arch):
        tables = orig(arch)
        if target not in tables:
            return tables
        return {
            name: (set(fns) if name == target else {f for f in fns if f not in strip})
            for name, fns in tables.items()
        }

    bacc.get_activation_tables = pinned


def _build_program():
    import concourse.bass as bass
    import concourse.mybir as mybir
    import concourse.tile as tile
    from concourse import bacc
    from concourse.tile_rust import add_dep_helper
    from contextlib import ExitStack

    _pin_act_tables()

    f32 = mybir.dt.float32
    f16 = mybir.dt.float16
    bf16 = mybir.dt.bfloat16
    Ln = mybir.ActivationFunctionType.Ln
    Exp = mybir.ActivationFunctionType.Exp
    Alu = mybir.AluOpType

    nc = bacc.Bacc(
        "TRN2", target_bir_lowering=False, debug=False, num_devices=NCORES
    )

    xt_d = nc.dram_tensor("xt", [IN, BS], bf16, kind="ExternalInput")
    wh_d = nc.dram_tensor("wh", [IN, OH], bf16, kind="ExternalInput")
    uh_d = nc.dram_tensor("uh", [IN, OH], f16, kind="ExternalInput")
    t_d = nc.dram_tensor("tt", [1], f32, kind="ExternalInput")
    out_d = nc.dram_tensor("out", [BS, OH], bf16, kind="ExternalOutput")

    replica_groups = [[0, 1], [2, 3], [4, 5], [6, 7]]
    NCH = len(CHUNKS)
    cbounds = []
    s = 0
    for csz in CHUNKS:
        cbounds.append((s, s + csz))
        s += csz
    assert s == KT
    NG = len(GROUPS)
    gbounds = []
    s = 0
    for gsz in GROUPS:
        gbounds.append((s, s + gsz))
        s += gsz
    assert s == KT
    NXT = KT // XTC

    with tile.TileContext(nc) as tc, ExitStack() as ctx:
        dram = ctx.enter_context(tc.tile_pool(name="dram", bufs=1, space="DRAM"))
        singles = ctx.enter_context(tc.tile_pool(name="singles", bufs=1))
        lpool = ctx.enter_context(tc.tile_pool(name="lpool", bufs=3))
        outp = ctx.enter_context(tc.tile_pool(name="outp", bufs=4))
        psum = ctx.enter_context(tc.tile_pool(name="psum", bufs=1, space="PSUM"))

        # 1/T broadcast to all partitions.
        t_sb = singles.tile([P, 1], f32)
        t_ap = t_d.ap()
        nc.sync.dma_start(
            out=t_sb, in_=bass.AP(tensor=t_ap.tensor, offset=0, ap=[[0, P], [1, 1]])
        )
        invt = singles.tile([P, 1], f32)
        nc.vector.reciprocal(invt, t_sb)

        zero_t = singles.tile([P, 1], f32)
        nc.vector.memset(zero_t, 0.0)
        one_t = singles.tile([P, 1], f32)
        nc.vector.memset(one_t, 1.0)

        # PE clock warmup: dummy accumulation group on a bank that real work
        # only reaches much later.
        dumm_l = singles.tile([P, P], bf16)
        dumm_r = singles.tile([P, OH], bf16)
        nc.vector.memset(dumm_l, 0.0)
        nc.vector.memset(dumm_r, 0.0)

        # CC stream warmup: dummy AllGather fired as early as possible.
        ccw_in = dram.tile([P, 1], f32, name="ccw_in", tag="ccw_in")
        ccw_out = dram.tile([2, P, 1], f32, name="ccw_out", tag="ccw_out")
        nc.sync.dma_start(out=ccw_in, in_=zero_t)
        nc.gpsimd.collective_compute(
            "AllGather",
            Alu.bypass,
            replica_groups=replica_groups,
            ins=[ccw_in.opt()],
            outs=[ccw_out.opt()],
        )

        # Resident tensors.
        xt_all = singles.tile([P, KT, BS], bf16)
        e_all = singles.tile([P, KT, OH], bf16)
        u_all = singles.tile([P, KT, OH], f16)
        w_all = singles.tile([P, KT, OH], bf16)
        sums = singles.tile([P, KT, 1], f32)
        invr = singles.tile([P, KT], f32)

        cc_in = [
            dram.tile([P, gsz], f32, name=f"cc_in{g}", tag=f"cc_in{g}")
            for g, gsz in enumerate(GROUPS)
        ]
        cc_out = [
            dram.tile([2, P, gsz], f32, name=f"cc_out{g}", tag=f"cc_out{g}")
            for g, gsz in enumerate(GROUPS)
        ]

        ps_tiles = [
            psum.tile([P, OH], f32, tag=f"ps{mb}", name=f"ps{mb}")
            for mb in range(MBT)
        ]

        for i in range(N_WARM_MM):
            nc.tensor.matmul(
                ps_tiles[MBT - 1][:],
                lhsT=dumm_l[:],
                rhs=dumm_r[:],
                start=(i == 0),
                stop=(i == N_WARM_MM - 1),
            )

        # All bulk loads ride the gpsimd hardware queue in one explicit FIFO
        # order: [u0 w0 u1 w1 u2 w2 xt0 u3 w3 xt1 ...].  Queue position IS
        # the pacing: u/w chunks lead, each xt chunk slots in two chunks
        # later.  The sync queue carries nothing bulky so the tiny
        # latency-critical cc DMAs never sit behind megabytes of FIFO.
        def u_load(kb):
            ks, ke = cbounds[kb]
            nc.gpsimd.dma_start(
                out=u_all[:, ks:ke, :],
                in_=uh_d[ks * P : ke * P, :].rearrange("(g p) c -> p g c", p=P),
            )
            nc.gpsimd.dma_start(
                out=w_all[:, ks:ke, :],
                in_=wh_d[ks * P : ke * P, :].rearrange("(g p) c -> p g c", p=P),
            )

        nxt_ld = 0
        for kb in range(NCH):
            u_load(kb)
            if kb >= 2 and nxt_ld < NXT:
                base = nxt_ld * XTC * P
                nc.gpsimd.dma_start(
                    out=xt_all[:, nxt_ld * XTC : (nxt_ld + 1) * XTC, :],
                    in_=xt_d[base : base + XTC * P, :].rearrange(
                        "(g p) b -> p g b", p=P
                    ),
                )
                nxt_ld += 1
        while nxt_ld < NXT:
            base = nxt_ld * XTC * P
            nc.gpsimd.dma_start(
                out=xt_all[:, nxt_ld * XTC : (nxt_ld + 1) * XTC, :],
                in_=xt_d[base : base + XTC * P, :].rearrange("(g p) b -> p g b", p=P),
            )
            nxt_ld += 1

        ln_a = {}  # chunk idx -> first Ln instruction (xt pacing anchor)
        reduces = {}  # chunk idx -> reduce instruction (finish skew anchor)

        def chunk_compute(kb):
            ks, ke = cbounds[kb]
            csz = ke - ks
            l_t = lpool.tile([P, csz, OH], f32, tag=f"l{csz}", name=f"l{csz}_t")
            # l = ln(1 - (1-tiny)*2^-14 * v')  ==  ln((1-tiny)*u + tiny) < 0
            ln_a[kb] = nc.scalar.activation(
                l_t, u_all[:, ks:ke, :], Ln, bias=one_t[:], scale=-(1.0 - TINY) / VSCALE
            )
            # m = ln(-l) = -gumbel
            nc.scalar.activation(l_t, l_t, Ln, bias=zero_t[:], scale=-1.0)
            eng = nc.vector if kb < VEC_SUB_CHUNKS else nc.gpsimd
            eng.tensor_sub(l_t, w_all[:, ks:ke, :], l_t)
            nc.scalar.activation(
                e_all[:, ks:ke, :], l_t, Exp, bias=zero_t[:], scale=invt[:]
            )
            reduces[kb] = nc.vector.tensor_reduce(
                sums[:, ks:ke, :],
                e_all[:, ks:ke, :],
                mybir.AxisListType.X,
                Alu.add,
            )

        def exchange(g):
            gs, ge = gbounds[g]
            nc.sync.dma_start(out=cc_in[g], in_=sums[:, gs:ge, 0])
            nc.gpsimd.collective_compute(
                "AllGather",
                Alu.bypass,
                replica_groups=replica_groups,
                ins=[cc_in[g].opt()],
                outs=[cc_out[g].opt()],
            )

        def finish(g, skew_kb):
            gs, ge = gbounds[g]
            gsz = ge - gs
            both = singles.tile([P, 2, gsz], f32, name=f"both{g}", tag=f"both{g}")
            nc.sync.dma_start(
                out=both, in_=cc_out[g][:].rearrange("g p k -> p g k")
            )
            tot = singles.tile([P, gsz], f32, name=f"tot{g}", tag=f"tot{g}")
            add_i = nc.vector.tensor_add(tot, both[:, 0, :], both[:, 1, :])
            if skew_kb is not None and skew_kb in reduces:
                add_dep_helper(
                    add_i.ins, reduces[skew_kb].ins, reason="finish skew"
                )
            nc.vector.reciprocal(invr[:, gs:ge], tot)
            for ki in range(gs, ge):
                nc.vector.tensor_scalar_mul(
                    e_all[:, ki, :], e_all[:, ki, :], invr[:, ki : ki + 1]
                )
            for ki in range(gs, ge):
                for mb in range(MBT):
                    nc.tensor.matmul(
                        ps_tiles[mb][:],
                        lhsT=xt_all[:, ki, mb * P : (mb + 1) * P],
                        rhs=e_all[:, ki, :],
                        start=(ki == 0),
                        stop=(ki == KT - 1),
                    )

        done_k = 0
        next_g = 0
        for kb in range(NCH):
            chunk_compute(kb)
            done_k = cbounds[kb][1]
            while next_g < NG and gbounds[next_g][1] <= done_k:
                exchange(next_g)
                finish(next_g, min(kb + 1, NCH - 1))
                next_g += 1
        assert next_g == NG

        # Drain PSUM (f32 -> bf16) and store.
        for mb in range(MBT):
            o_t = outp.tile([P, OH], bf16, tag="o")
            nc.vector.tensor_copy(o_t, ps_tiles[mb][:])
            nc.sync.dma_start(out=out_d[mb * P : (mb + 1) * P, :], in_=o_t)

    nc.compile()
    return nc


def kernel(x, weight, uniform, T):
    global _PROGRAM, LAST_RESULT
    import ml_dtypes
    from concourse.bass_utils import run_bass_kernel_spmd

    if _PROGRAM is None:
        _PROGRAM = _build_program()
    nc = _PROGRAM

    bf = ml_dtypes.bfloat16
    x = np.asarray(x, dtype=np.float32)
    weight = np.asarray(weight, dtype=np.float32)
    uniform = np.asarray(uniform, dtype=np.float32)
    T = np.ascontiguousarray(np.asarray(T, dtype=np.float32)).reshape([1])

    xt = np.ascontiguousarray(x.T.astype(bf))  # [IN, B] bf16
    wb = weight.astype(bf)
    # v' = (1-u) * 2^14 in fp16: full relative precision at the u->1 tail
    # (which dominates the softmax) without any fp16 subnormals.
    vq = ((1.0 - uniform.astype(np.float64)) * VSCALE).astype(np.float16)
    vq = np.maximum(vq, np.float16(2.0**-10))
    in_maps = []
    for c in range(NCORES):
        p, q = c // GO, c % GO
        in_maps.append(
            {
                "xt": np.ascontiguousarray(xt[:, p * BS : (p + 1) * BS]),
                "wh": np.ascontiguousarray(wb[:, q * OH : (q + 1) * OH]),
                "uh": np.ascontiguousarray(vq[:, q * OH : (q + 1) * OH]),
                "tt": T,
            }
        )

    res = run_bass_kernel_spmd(nc, in_maps, core_ids=list(range(NCORES)))
    LAST_RESULT = res

    out = np.empty((B, OUT), dtype=np.float32)
    for c in range(NCORES):
        p, q = c // GO, c % GO
        out[p * BS : (p + 1) * BS, q * OH : (q + 1) * OH] = res.results[c][
            "out"
        ].astype(np.float32)
    return out
